# revision 12
# baseline (speedup 1.0000x reference)
"""AttentionGRUDecoder Trainium2 kernel.

8 cores = 2 batch-groups (B=256 each) x 4-way model shard within each
group. Combine (attention_combine) is K-sharded over (h,attn)-quarters,
GRU gates M-sharded over h-quarters, encoder slice d-quartered. Two
group collectives per step: AllReduce of combine partials and a
zero-padded AllReduce regathering h_new quarters.

Folding: dec_in(t) = h(t) @ Wo.T + bo for t>=1, so Wo folds into Wa and
Wc (Wa_eff = WaD@Wo + WaH, M1 = WcD@Wo); out_tok is produced per step
from the regathered h off the critical path of the recurrence state.

Layout: activations transposed [feature-part, batch-free]. Linear
matmuls keep weights stationary (f32r: full rate at moving dim 256) and
stream activations. The einsum bs,bsd->bd runs as block-diagonal bf16
matmuls (batch-pair x 64 seq per 128-contraction tile, 32 batches per
output block for 32-aligned PSUM bases).

The per-core quarter enters the one program (SPMD) as data: a "qoff"
input register drives dynamic sync-engine DMA offsets.
"""

import numpy as np
from contextlib import ExitStack

import orjson

import concourse.bass as bass
import concourse.bacc as bacc
import concourse.tile as tile
from concourse import mybir
from concourse import bass_utils
from concourse.masks import make_identity
from concourse.vector_clock import ScopedClock, VectorClock

F32 = mybir.dt.float32
F32R = mybir.dt.float32r
BF16 = mybir.dt.bfloat16
AF = mybir.ActivationFunctionType
ALU = mybir.AluOpType
AX = mybir.AxisListType

B, S, H, O = 512, 64, 1024, 1024
T = 128
NCORES = 8

# --- compiler workarounds: this walrus build leaves ~1 free sync-wait slot
# per instruction struct; move extra waits onto injected same-engine NoOps
# (engine program order makes them equivalent). ---

def _rewrite_bir_json(bir_json: bytes) -> bytes:
    d = orjson.loads(bir_json)
    ctr = 0
    for fn in d.get("functions", []):
        for bb in fn.get("blocks", []):
            insts = bb.get("instructions")
            if not insts:
                continue
            out = []
            changed = False
            for inst in insts:
                si = inst.get("sync_info")
                ow = (si or {}).get("on_wait") or []
                if len(ow) > 1:
                    for w in ow[:-1]:
                        ctr += 1
                        out.append({
                            "debug": inst.get("debug", 0),
                            "engine": inst["engine"],
                            "ins": [], "outs": [],
                            "name": f"waitfix-{ctr}",
                            "opcode": "NoOp",
                            "sync_info": {"on_update": [], "on_wait": [w]},
                        })
                    si["on_wait"] = ow[-1:]
                    changed = True
                out.append(inst)
            if changed:
                bb["instructions"] = out
    return orjson.dumps(d)


_patched = False

def _install_patches():
    global _patched
    if _patched:
        return
    import concourse.bass_utils as bu
    import concourse.bass2jax as b2j
    orig = bu.compile_bir_kernel

    def patched(bir_json, tmpdir, neff_name="file.neff"):
        return orig(_rewrite_bir_json(bytes(bir_json)), tmpdir, neff_name=neff_name)

    bu.compile_bir_kernel = patched
    b2j.compile_bir_kernel = patched
    _patched = True


class PatchedTileContext(tile.TileContext):
    """Kernel-tail drain has no free wait slots: chunk its sem waits onto
    preceding sync-engine NoOps."""

    def _drain_and_barrier(self, tick_clock, wait_clock):
        gc = tick_clock.global_clock
        n = len(gc)
        for start in range(0, n, 4):
            vec = [gc[i] if start <= i < start + 4 else 0 for i in range(n)]
            if not any(vec):
                continue
            nop_inst = self.nc.sync.nop(nofuse=True, hint="drain_waits")
            wait_clock.add_sem_waits(nop_inst.ins, ScopedClock({None: VectorClock(vec)}))
        self.nc.sync.drain()
        self.nc.all_engine_barrier()
        assert self.sems is not None
        popped = self.nc._tile_sem_poison_stack.pop()
        assert popped is self._sem_poison
        self.nc.clear_and_free_semaphores(list(self.sems.allocated().values()))
        self.nc.all_engine_barrier()


def _build_program():
    nc = bacc.Bacc("TRN2", target_bir_lowering=False, debug=False,
                   num_devices=NCORES)

    def inp(name, shape, dt=F32):
        return nc.dram_tensor(name, shape, dt, kind="ExternalInput").ap()

    encp = inp("encp", [128, 128, 256])
    h0T = inp("h0T", [8, 128, 256])
    waeffT = inp("waeffT", [8, 128, 64])
    waHT = inp("waHT", [8, 128, 64])
    m1T = inp("m1T", [2, 128, 1024])
    wcaT = inp("wcaT", [2, 128, 1024])
    wihT = inp("wihT", [8, 128, 768])
    whhT = inp("whhT", [8, 128, 768])
    woT = inp("woT", [8, 128, 256])
    biases = inp("biases", [128, 32])
    h0q = inp("h0q", [128, 512])

    outq = nc.dram_tensor("outq", [T, 128, 512], F32, kind="ExternalOutput").ap()
    hfin = nc.dram_tensor("hfin", [128, 2048], F32, kind="ExternalOutput").ap()

    groups = [[0, 1, 2, 3], [4, 5, 6, 7]]

    with PatchedTileContext(nc) as tc, ExitStack() as ctx:
        wpool = ctx.enter_context(tc.tile_pool(name="weights", bufs=1))
        spool = ctx.enter_context(tc.tile_pool(name="state", bufs=1))
        work = ctx.enter_context(tc.tile_pool(name="work", bufs=1))
        big = ctx.enter_context(tc.tile_pool(name="big", bufs=1))
        stage = ctx.enter_context(tc.tile_pool(name="stage", bufs=1))
        dram = ctx.enter_context(tc.tile_pool(name="dram", bufs=1, space="DRAM"))

        psA = ctx.enter_context(tc.tile_pool(name="psA", bufs=1, space="PSUM"))
        psB = ctx.enter_context(tc.tile_pool(name="psB", bufs=1, space="PSUM"))
        psC1 = ctx.enter_context(tc.tile_pool(name="psC1", bufs=1, space="PSUM"))
        psC2 = ctx.enter_context(tc.tile_pool(name="psC2", bufs=2, space="PSUM"))
        psD = ctx.enter_context(tc.tile_pool(name="psD", bufs=1, space="PSUM"))

        enc_bf = spool.tile([128, 128 * 256], BF16)
        w32 = spool.tile([128, 8 * 512], BF16)
        hT = spool.tile([128, 2048], F32)
        hTr = spool.tile([128, 2048], F32R)
        hqs = spool.tile([128, 512], F32R)      # own h quarter (rounded)
        hqf = spool.tile([128, 512], F32)        # own h quarter
        gruT = spool.tile([128, 2048], F32R)
        ident = spool.tile([128, 128], F32)
        bias_sb = spool.tile([128, 32], F32)

        w_waeff = wpool.tile([128, 8 * 64], F32R)
        w_waH = wpool.tile([128, 8 * 64], F32R)
        w_m1 = wpool.tile([128, 2 * 1024], F32R)
        w_wca = wpool.tile([128, 2 * 1024], F32R)
        w_wih = wpool.tile([128, 8 * 768], F32R)
        w_whh = wpool.tile([128, 8 * 768], F32R)
        w_wo = wpool.tile([128, 8 * 256], F32R)

        make_identity(nc, ident[:])
        nc.gpsimd.memset(w32[:], 0.0)
        nc.sync.dma_start(bias_sb[:], biases[:])

        def load_round(dst, src, n, cols):
            for j in range(n):
                for h2 in range(0, cols, 512):
                    w = min(512, cols - h2)
                    st = stage.tile([128, 512], F32, tag="wstage")
                    nc.sync.dma_start(st[:, 0:w], src[j][:, h2:h2 + w])
                    nc.vector.tensor_copy(
                        dst[:, j * cols + h2:j * cols + h2 + w], st[:, 0:w])

        load_round(w_waeff, waeffT, 8, 64)
        load_round(w_waH, waHT, 8, 64)
        load_round(w_m1, m1T, 2, 1024)
        load_round(w_wca, wcaT, 2, 1024)
        load_round(w_wih, wihT, 8, 768)
        load_round(w_whh, whhT, 8, 768)
        load_round(w_wo, woT, 8, 256)

        for p in range(128):
            st = stage.tile([128, 256], F32, tag="estage")
            nc.sync.dma_start(st[:], encp[p])
            nc.vector.tensor_copy(enc_bf[:, p * 256:(p + 1) * 256], st[:])

        for j in range(8):
            nc.sync.dma_start(hT[:, j * 256:(j + 1) * 256], h0T[j])
        nc.vector.tensor_copy(hTr[:], hT[:])
        nc.sync.dma_start(hqf[:], h0q[:])
        nc.vector.tensor_copy(hqs[:], hqf[:])

        gb_in = dram.tile([128, 2048], F32)
        gb_out = dram.tile([128, 2048], F32)
        hb_in = dram.tile([128, 2048], F32)
        hb_out = dram.tile([128, 2048], F32)

        def step(t0, tidx):
            wa = w_waH if t0 else w_waeff
            bacol = 1 if t0 else 0
            bccol = 10 if t0 else 2

            # attention logits [64(att), 256(b)]
            ps_l = psD.tile([128, 256], F32, tag="d")
            for k in range(8):
                nc.tensor.matmul(ps_l[0:64, :], wa[:, k * 64:(k + 1) * 64],
                                 hTr[:, k * 256:(k + 1) * 256],
                                 start=(k == 0), stop=(k == 7))
            lT = work.tile([64, 256], F32, tag="lT")
            nc.scalar.activation(lT[:], ps_l[0:64, :], AF.Identity,
                                 bias=bias_sb[0:64, bacol:bacol + 1])

            # -> batch-major [128(b), 2x64(s)]
            lB = work.tile([128, 128], F32, tag="lB")
            for h2 in range(2):
                ps_t = psD.tile([128, 256], F32, tag="d")
                nc.tensor.transpose(ps_t[0:128, 0:64],
                                    lT[:, h2 * 128:(h2 + 1) * 128],
                                    ident[0:64, 0:64])
                nc.scalar.activation(lB[:, h2 * 64:(h2 + 1) * 64],
                                     ps_t[0:128, 0:64], AF.Copy)

            # softmax over s per 64-col half
            sfx = work.tile([128, 8], F32, tag="sfx")
            mx, sm, rs = sfx[:, 0:2], sfx[:, 2:4], sfx[:, 4:6]
            wB = work.tile([128, 128], F32, tag="wB")
            nc.vector.tensor_reduce(
                mx.rearrange("p (h c) -> p h c", c=1),
                lB[:].rearrange("p (h s) -> p h s", s=64),
                axis=AX.X, op=ALU.max)
            nc.vector.tensor_scalar_mul(mx, mx, -1.0)
            for h2 in range(2):
                nc.scalar.activation(wB[:, h2 * 64:(h2 + 1) * 64],
                                     lB[:, h2 * 64:(h2 + 1) * 64], AF.Exp,
                                     bias=mx[:, h2:h2 + 1])
            nc.vector.tensor_reduce(
                sm.rearrange("p (h c) -> p h c", c=1),
                wB[:].rearrange("p (h s) -> p h s", s=64),
                axis=AX.X, op=ALU.add)
            nc.vector.reciprocal(rs, sm)
            for h2 in range(2):
                nc.scalar.activation(wB[:, h2 * 64:(h2 + 1) * 64],
                                     wB[:, h2 * 64:(h2 + 1) * 64], AF.Copy,
                                     scale=rs[:, h2:h2 + 1])

            # transpose back: [(h2,s), b_local]
            ps_wT = psD.tile([128, 256], F32, tag="d")
            nc.tensor.transpose(ps_wT[0:128, 0:128], wB[:], ident[:])

            # scatter into block-diagonal bf16 stationary
            for blk in range(8):
                h2 = blk // 4
                base = (blk % 4) * 32
                src = ps_wT[h2 * 64:h2 * 64 + 64, :]
                c0 = blk * 512
                nc.vector.tensor_copy(w32[0:64, c0:c0 + 511:34],
                                      src[:, base:base + 32:2])
                nc.vector.tensor_copy(w32[64:128, c0 + 1:c0 + 512:34],
                                      src[:, base + 1:base + 32:2])

            # attention applied, blocks of 32 batches
            attn_B = work.tile([128, 512], F32, tag="attnB")
            for half in range(2):
                ps_a = psC1.tile([128, 256], F32, tag="c")
                for bq in range(4):
                    blk = half * 4 + bq
                    for j in range(16):
                        pr = blk * 16 + j
                        nc.tensor.matmul(
                            ps_a[bq * 32:(bq + 1) * 32, :],
                            w32[:, blk * 512 + j * 32: blk * 512 + (j + 1) * 32],
                            enc_bf[:, pr * 256:(pr + 1) * 256],
                            start=(j == 0), stop=(j == 15),
                            tile_position=(0, bq * 32))
                nc.scalar.activation(attn_B[:, half * 256:(half + 1) * 256],
                                     ps_a[:], AF.Copy)

            # -> T layout f32r [d, b]
            attnT = work.tile([128, 512], F32R, tag="attnT")
            for dj in range(2):
                for half in range(2):
                    ps_t2 = psD.tile([128, 256], F32, tag="d")
                    nc.tensor.transpose(
                        ps_t2[0:128, 0:128],
                        attn_B[:, half * 256 + dj * 128: half * 256 + (dj + 1) * 128],
                        ident[:])
                    nc.vector.tensor_copy(
                        attnT[:, dj * 256 + half * 128: dj * 256 + (half + 1) * 128],
                        ps_t2[0:128, 0:128])

            # combine partials (K-shard: this core's h quarter + attn quarter)
            gsb = big.tile([128, 2048], F32, tag="gx")
            for half in range(2):
                ps_g = psA.tile([128, 1024], F32, tag="a")
                for m in range(4):
                    mm = half * 4 + m
                    first = True
                    if not t0:
                        for kk in range(2):
                            nc.tensor.matmul(
                                ps_g[:, m * 256:(m + 1) * 256],
                                w_m1[:, kk * 1024 + mm * 128: kk * 1024 + (mm + 1) * 128],
                                hqs[:, kk * 256:(kk + 1) * 256],
                                start=first, stop=False)
                            first = False
                    for kk in range(2):
                        nc.tensor.matmul(
                            ps_g[:, m * 256:(m + 1) * 256],
                            w_wca[:, kk * 1024 + mm * 128: kk * 1024 + (mm + 1) * 128],
                            attnT[:, kk * 256:(kk + 1) * 256],
                            start=first, stop=(kk == 1))
                        first = False
                nc.scalar.activation(gsb[:, half * 1024:(half + 1) * 1024],
                                     ps_g[:], AF.Copy)

            nc.sync.dma_start(gb_in[:], gsb[:])
            nc.gpsimd.collective_compute(
                "AllReduce", ALU.add, replica_groups=groups,
                ins=[gb_in.opt()], outs=[gb_out.opt()])
            graw = big.tile([128, 2048], F32, tag="gx")
            nc.sync.dma_start(graw[:], gb_out[:])
            for m in range(8):
                nc.scalar.activation(gruT[:, m * 256:(m + 1) * 256],
                                     graw[:, m * 256:(m + 1) * 256], AF.Relu,
                                     bias=bias_sb[:, bccol + m:bccol + m + 1])

            # GRU gates (M-shard quarter)
            ps_rz = psB.tile([128, 1024], F32, tag="b")
            ps_gin = psC2.tile([128, 512], F32, tag="c2")
            ps_ghn = psC2.tile([128, 512], F32, tag="c2")
            for m in range(4):
                for k in range(8):
                    nc.tensor.matmul(ps_rz[:, m * 256:(m + 1) * 256],
                                     w_wih[:, k * 768 + m * 128: k * 768 + (m + 1) * 128],
                                     gruT[:, k * 256:(k + 1) * 256],
                                     start=(k == 0), stop=False)
                for k in range(8):
                    nc.tensor.matmul(ps_rz[:, m * 256:(m + 1) * 256],
                                     w_whh[:, k * 768 + m * 128: k * 768 + (m + 1) * 128],
                                     hTr[:, k * 256:(k + 1) * 256],
                                     start=False, stop=(k == 7))
            for m in range(2):
                for k in range(8):
                    nc.tensor.matmul(ps_gin[:, m * 256:(m + 1) * 256],
                                     w_wih[:, k * 768 + (4 + m) * 128: k * 768 + (5 + m) * 128],
                                     gruT[:, k * 256:(k + 1) * 256],
                                     start=(k == 0), stop=(k == 7))
                for k in range(8):
                    nc.tensor.matmul(ps_ghn[:, m * 256:(m + 1) * 256],
                                     w_whh[:, k * 768 + (4 + m) * 128: k * 768 + (5 + m) * 128],
                                     hTr[:, k * 256:(k + 1) * 256],
                                     start=(k == 0), stop=(k == 7))

            rz = big.tile([128, 1024], F32, tag="rz")
            for m in range(4):
                nc.scalar.activation(rz[:, m * 256:(m + 1) * 256],
                                     ps_rz[:, m * 256:(m + 1) * 256], AF.Sigmoid,
                                     bias=bias_sb[:, 18 + m:19 + m])
            t1 = work.tile([128, 512], F32, tag="t1")
            g2 = work.tile([128, 512], F32, tag="g2")
            for m in range(2):
                nc.scalar.activation(t1[:, m * 256:(m + 1) * 256],
                                     ps_ghn[:, m * 256:(m + 1) * 256], AF.Identity,
                                     bias=bias_sb[:, 24 + m:25 + m])
                nc.scalar.activation(g2[:, m * 256:(m + 1) * 256],
                                     ps_gin[:, m * 256:(m + 1) * 256], AF.Identity,
                                     bias=bias_sb[:, 22 + m:23 + m])
            nc.vector.tensor_mul(t1[:], rz[:, 0:512], t1[:])
            nc.vector.tensor_add(t1[:], t1[:], g2[:])
            nt = work.tile([128, 512], F32, tag="nt")
            nc.scalar.activation(nt[:], t1[:], AF.Tanh)

            nc.vector.tensor_sub(g2[:], hqf[:], nt[:])
            nc.vector.tensor_mul(g2[:], rz[:, 512:1024], g2[:])
            nc.vector.tensor_add(hqf[:], nt[:], g2[:])
            nc.vector.tensor_copy(hqs[:], hqf[:])

            # regather h within group: mask-padded AllReduce (all static)
            hstg = big.tile([128, 2048], F32, tag="gx")
            for j in range(4):
                nc.vector.tensor_scalar_mul(hstg[:, j * 512:(j + 1) * 512],
                                            hqf[:], bias_sb[:, 28 + j:29 + j])
            nc.sync.dma_start(hb_in[:], hstg[:])
            nc.gpsimd.collective_compute(
                "AllReduce", ALU.add, replica_groups=groups,
                ins=[hb_in.opt()], outs=[hb_out.opt()])
            nc.sync.dma_start(hT[:], hb_out[:])
            nc.vector.tensor_copy(hTr[:], hT[:])

            # out_tok quarter from regathered h
            ps_o = psC2.tile([128, 512], F32, tag="c2")
            for m in range(2):
                for k in range(8):
                    nc.tensor.matmul(ps_o[:, m * 256:(m + 1) * 256],
                                     w_wo[:, k * 256 + m * 128: k * 256 + (m + 1) * 128],
                                     hTr[:, k * 256:(k + 1) * 256],
                                     start=(k == 0), stop=(k == 7))
            osb = work.tile([128, 512], F32, tag="osb")
            for m in range(2):
                nc.scalar.activation(osb[:, m * 256:(m + 1) * 256],
                                     ps_o[:, m * 256:(m + 1) * 256], AF.Identity,
                                     bias=bias_sb[:, 26 + m:27 + m])
            if isinstance(tidx, int):
                nc.sync.dma_start(outq[tidx, :, :], osb[:])
            else:
                nc.sync.dma_start(outq[bass.ds(tidx, 1), :, :], osb[:])

        for t in range(T):
            step(t == 0, t)

        nc.sync.dma_start(hfin[:], hT[:])

    nc.compile()
    return nc


_prog_cache = {}


def kernel(**inputs):
    enc = np.asarray(inputs["encoder_outputs"], np.float32)
    hidden = np.asarray(inputs["hidden"], np.float32)
    Wa = np.asarray(inputs["Wa"], np.float64)
    ba = np.asarray(inputs["ba"], np.float64)
    Wc = np.asarray(inputs["Wc"], np.float64)
    bc = np.asarray(inputs["bc"], np.float64)
    W_ih = np.asarray(inputs["W_ih"], np.float64)
    W_hh = np.asarray(inputs["W_hh"], np.float64)
    b_ih = np.asarray(inputs["b_ih"], np.float64)
    b_hh = np.asarray(inputs["b_hh"], np.float64)
    Wo = np.asarray(inputs["Wo"], np.float64)
    bo = np.asarray(inputs["bo"], np.float64)

    WaD, WaH = Wa[:, :O], Wa[:, O:]
    WcD, WcA = Wc[:, :O], Wc[:, O:]
    Wa_eff = WaD @ Wo + WaH
    ba_eff = ba + WaD @ bo
    M1 = WcD @ Wo
    bc_eff = bc + WcD @ bo

    def t8(Wt, n, cols):
        return np.ascontiguousarray(np.asarray(Wt, np.float32).reshape(n, 128, cols))

    in_maps = []
    for c in range(NCORES):
        g, q = c // 4, c % 4
        bg = slice(256 * g, 256 * (g + 1))
        hq = slice(256 * q, 256 * (q + 1))
        dq = slice(256 * q, 256 * (q + 1))
        gate_rows = np.r_[256 * q:256 * (q + 1),
                          1024 + 256 * q:1024 + 256 * (q + 1),
                          2048 + 256 * q:2048 + 256 * (q + 1)]
        oq = slice(256 * q, 256 * (q + 1))

        e = enc[bg, :, dq].reshape(128, 128, 256)
        h0T = np.ascontiguousarray(hidden[0, bg, :].T.reshape(8, 128, 256),
                                   np.float32)

        bias = np.zeros((128, 32), np.float32)
        bias[0:64, 0] = ba_eff
        bias[0:64, 1] = ba
        bias[:, 2:10] = bc_eff.reshape(8, 128).T
        bias[:, 10:18] = bc.reshape(8, 128).T
        bias[:, 18:22] = (b_ih + b_hh)[gate_rows[:512]].reshape(4, 128).T
        bias[:, 22:24] = b_ih[gate_rows[512:]].reshape(2, 128).T
        bias[:, 24:26] = b_hh[gate_rows[512:]].reshape(2, 128).T
        bias[:, 26:28] = bo[oq].reshape(2, 128).T
        bias[:, 28 + q] = 1.0

        in_maps.append({
            "encp": np.ascontiguousarray(e, np.float32),
            "h0T": h0T,
            "waeffT": t8(Wa_eff.T, 8, 64),
            "waHT": t8(WaH.T, 8, 64),
            "m1T": t8(M1[:, hq].T, 2, 1024),
            "wcaT": t8(WcA[:, dq].T, 2, 1024),
            "wihT": t8(W_ih[gate_rows, :].T, 8, 768),
            "whhT": t8(W_hh[gate_rows, :].T, 8, 768),
            "woT": t8(Wo[oq, :].T, 8, 256),
            "biases": bias,
            "h0q": np.ascontiguousarray(
                hidden[0, bg, 256 * q:256 * (q + 1)].T.reshape(2, 128, 256)
                .transpose(1, 0, 2).reshape(128, 512), np.float32),
        })

    if "nc" not in _prog_cache:
        _install_patches()
        _prog_cache["nc"] = _build_program()
    nc = _prog_cache["nc"]
    r = bass_utils.run_bass_kernel_spmd(nc, in_maps, core_ids=list(range(NCORES)))

    out = np.empty((B, T, O), np.float32)
    for c in range(NCORES):
        g, q = c // 4, c % 4
        oq_res = r.results[c]["outq"]  # [t, o_in_tile, (m, b)]
        out[256 * g:256 * (g + 1), :, 256 * q:256 * (q + 1)] = (
            oq_res.reshape(T, 128, 2, 256).transpose(3, 0, 2, 1).reshape(256, T, 256))
    hf = np.empty((B, H), np.float32)
    for g in range(2):
        ht = r.results[4 * g]["hfin"]  # [p, (j, b)]
        hf[256 * g:256 * (g + 1), :] = (
            ht.reshape(128, 8, 256).transpose(2, 1, 0).reshape(256, H))
    return out, hf[None]
